# revision 7
# baseline (speedup 1.0000x reference)
"""Trainium2 Bass kernel for a 3D AttentionBlock:
GroupNorm -> 1x1x1-conv QKV -> (2x2x2 avg-pooled K/V) attention -> proj -> residual.

SPMD across 8 NeuronCores: core = (batch b, spatial quarter). Each core computes
the full block for 3456 of the 13824 spatial positions of one batch element; the
pooled K/V (1728 positions) are computed redundantly per core from the full x[b].
No cross-core communication.

A host-side np.roll of x[b] along the flattened spatial dim by the quarter offset
(a whole number of h-plane pairs) makes the program SPMD-uniform: every core's
program processes query columns [0, 3456). GroupNorm stats are permutation
invariant, the 2x2x2 pooling structure is preserved by the 6-plane rotation, and
softmax/attention are invariant to the induced permutation of key positions.

Algebraic folds:
 - GroupNorm affine (data-dependent per-channel scale s_c / shift t_c) is folded
   into the QKV weights on device: W' = W .* s_c (per input channel), b' = W@t + b.
 - avg-pooling commutes with the 1x1 conv: K/V are computed from pooled(x).
   The 1/8 pool mean is folded into the K/V weight scaling.
 - the attention scale (hd^-0.5) is folded into W_q/b_q on the host.
 - K is zero-padded 1728 -> 1792 (14 full 128-wide m-tiles); this adds exp(0)=1
   to every softmax denominator 64 times, which is subtracted exactly; padded V
   rows are zero so the AV matmul is unaffected.

PE usage: scores are computed transposed, S^T[m,n] = k^T q, with the 4 heads
row-tiled (tile_position=(32h,0), K=32 each). exp runs on ScalarE directly from
PSUM in 3-bank groups (this is the kernel's bottleneck: ~24M exps per core).
AV and the softmax-denominator matmuls are col-tiled per head
(tile_position=(0,32h)) accumulating over the 14 m-tiles in single PSUM banks.
Matmuls use float32r (1 cycle/row); probabilities and V are bf16.
"""

import numpy as np
import ml_dtypes
from contextlib import ExitStack

import concourse.bass as bass
import concourse.tile as tile
from concourse import mybir
from concourse.bacc import Bacc
from concourse.bass_utils import run_bass_kernel_spmd

F32 = mybir.dt.float32
F32R = mybir.dt.float32r
BF16 = mybir.dt.bfloat16
AF = mybir.ActivationFunctionType
ALU = mybir.AluOpType

C = 128            # channels
SP = 13824         # 24^3 spatial
NQ = SP // 4       # 3456 query columns per core
M = 1728           # pooled 12^3
MP = 1792          # padded to 14*128
NMT = MP // 128    # 14 m-tiles
NH = 4             # heads
HD = 32            # head dim
EPS = 1e-5
BLOCKS = [512] * 6 + [384]   # n-blocks covering NQ
XCH = 8                      # x DMA chunks
XCW = SP // XCH              # 1728 cols per chunk

_CACHE = {}


def _body(nc, ctx, tc, dram):
    x, wqkv, bqkv, wp, pb, gnw, gnb, gsum, gbr, ident, ones, out = dram

    const = ctx.enter_context(tc.tile_pool(name="const", bufs=1))
    sb = ctx.enter_context(tc.tile_pool(name="sb", bufs=1))
    work = ctx.enter_context(tc.tile_pool(name="work", bufs=2))
    ptp = ctx.enter_context(tc.tile_pool(name="ptp", bufs=3))
    stg = ctx.enter_context(tc.tile_pool(name="stg", bufs=2))
    ps = ctx.enter_context(tc.tile_pool(name="ps", bufs=1, space="PSUM"))

    dma = nc.default_dma_engine

    # ---------------- constants ----------------
    wq_t = const.tile([C, 3 * C], F32R)
    dma.dma_start(out=wq_t, in_=wqkv[:, :])
    bq_t = const.tile([C, 3], F32)
    dma.dma_start(out=bq_t, in_=bqkv[:, :])
    wp_t = const.tile([C, C], F32R)
    dma.dma_start(out=wp_t, in_=wp[:, :])
    pb_t = const.tile([C, 1], F32)
    dma.dma_start(out=pb_t, in_=pb[:, :])
    gnw_t = const.tile([C, 1], F32)
    dma.dma_start(out=gnw_t, in_=gnw[:, :])
    gnb_t = const.tile([C, 1], F32)
    dma.dma_start(out=gnb_t, in_=gnb[:, :])
    gsum_t = const.tile([C, 8], F32R)
    dma.dma_start(out=gsum_t, in_=gsum[:, :])
    gbr_t = const.tile([8, C], F32R)
    dma.dma_start(out=gbr_t, in_=gbr[:, :])
    ident_t = const.tile([C, C], BF16)
    dma.dma_start(out=ident_t, in_=ident[:, :])
    ones_t = const.tile([C, HD], BF16)
    dma.dma_start(out=ones_t, in_=ones[:, :])
    eps_t = const.tile([C, 1], F32)
    nc.vector.memset(eps_t, EPS)

    # ---------------- load x; per-channel stats; pooling ----------------
    x_sb = sb.tile([C, SP], F32R)
    stats = sb.tile([C, 32, 6], F32)
    xps = sb.tile([C, M], F32R)  # pooled *sums* (x8 of the mean)
    for ch in range(XCH):
        dma.dma_start(out=x_sb[:, ch * XCW:(ch + 1) * XCW],
                      in_=x[:, ch * XCW:(ch + 1) * XCW])
        for j in range(4):
            lo = ch * XCW + j * 432
            nc.vector.bn_stats(out=stats[:, ch * 4 + j, :], in_=x_sb[:, lo:lo + 432])
    for st in range(4):  # each step pools 6 h-planes (two DMA chunks)
        base = st * 3456
        xv = x_sb[:, base:base + 3456].rearrange(
            "p (h w d t) -> p h w d t", h=6, w=24, d=12, t=2)
        t1 = work.tile([C, 6, 24, 12], F32, tag="t1")
        nc.vector.tensor_tensor(out=t1, in0=xv[:, :, :, :, 0], in1=xv[:, :, :, :, 1],
                                op=ALU.add)
        t1v = t1.rearrange("p h (w t) d -> p h w t d", t=2)
        t2 = work.tile([C, 6, 12, 12], F32, tag="t2")
        nc.vector.tensor_tensor(out=t2, in0=t1v[:, :, :, 0, :], in1=t1v[:, :, :, 1, :],
                                op=ALU.add)
        t2v = t2.rearrange("p (h t) w d -> p h t w d", t=2)
        ov = xps[:, st * 432:(st + 1) * 432].rearrange("p (h w d) -> p h w d", h=3, w=12)
        nc.vector.tensor_tensor(out=ov, in0=t2v[:, :, 0, :, :], in1=t2v[:, :, 1, :, :],
                                op=ALU.add)

    # ---------------- GroupNorm stats -> per-channel scale/shift ----------------
    mv = sb.tile([C, 2], F32)
    nc.vector.bn_aggr(out=mv, in_=stats)
    m12 = sb.tile([C, 2], F32R)          # [mean_c, E[x^2]_c]
    nc.vector.tensor_copy(out=m12[:, 0:1], in_=mv[:, 0:1])
    nc.vector.tensor_tensor(out=m12[:, 1:2], in0=mv[:, 0:1], in1=mv[:, 0:1], op=ALU.mult)
    nc.vector.tensor_tensor(out=m12[:, 1:2], in0=m12[:, 1:2], in1=mv[:, 1:2], op=ALU.add)
    g_ps = ps.tile([8, 2], F32, tag="sums")
    nc.tensor.matmul(g_ps, gsum_t.bitcast(F32), m12.bitcast(F32), start=True, stop=True)
    g_sb = sb.tile([8, 2], F32R)
    nc.vector.tensor_copy(out=g_sb, in_=g_ps)
    bc_ps = ps.tile([C, 2], F32, tag="acc")
    nc.tensor.matmul(bc_ps, gbr_t.bitcast(F32), g_sb.bitcast(F32), start=True, stop=True)
    bc = sb.tile([C, 2], F32)           # [mu_g, E_g[x^2]] broadcast to channels
    nc.vector.tensor_copy(out=bc, in_=bc_ps)
    var_t = sb.tile([C, 1], F32)
    nc.vector.tensor_tensor(out=var_t, in0=bc[:, 0:1], in1=bc[:, 0:1], op=ALU.mult)
    nc.vector.tensor_tensor(out=var_t, in0=bc[:, 1:2], in1=var_t, op=ALU.subtract)
    sd_t = sb.tile([C, 1], F32)
    nc.scalar.activation(out=sd_t, in_=var_t, func=AF.Sqrt, bias=eps_t)
    r_t = sb.tile([C, 1], F32)
    nc.vector.reciprocal(out=r_t, in_=sd_t)
    s_t = sb.tile([C, 1], F32)          # s_c = gamma_c * rsqrt(var+eps)
    nc.vector.tensor_tensor(out=s_t, in0=r_t, in1=gnw_t, op=ALU.mult)
    s8_t = sb.tile([C, 1], F32)         # s_c / 8 (pool mean fold)
    nc.vector.tensor_scalar_mul(out=s8_t, in0=s_t, scalar1=0.125)
    tt_t = sb.tile([C, 1], F32R)         # t_c = beta_c - mu_c * s_c
    nc.vector.tensor_tensor(out=tt_t, in0=bc[:, 0:1], in1=s_t, op=ALU.mult)
    nc.vector.tensor_tensor(out=tt_t, in0=gnb_t, in1=tt_t, op=ALU.subtract)

    # ---------------- fold GN into QKV weights / biases ----------------
    wsc = sb.tile([C, 3 * C], F32R)
    nc.vector.tensor_scalar_mul(out=wsc[:, 0:C], in0=wq_t[:, 0:C], scalar1=s_t)
    nc.vector.tensor_scalar_mul(out=wsc[:, C:3 * C], in0=wq_t[:, C:3 * C], scalar1=s8_t)
    b_ps = ps.tile([C, 3], F32, tag="sums")
    for j in range(3):
        nc.tensor.matmul(b_ps[:, j:j + 1], wq_t[:, j * C:(j + 1) * C].bitcast(F32),
                         tt_t.bitcast(F32), start=True, stop=True)
    b_sb = sb.tile([C, 3], F32)
    nc.vector.tensor_tensor(out=b_sb, in0=b_ps, in1=bq_t, op=ALU.add)

    # ---------------- QKV ----------------
    q_sb = sb.tile([C, NQ], F32R)
    off = 0
    for w in BLOCKS:
        q_ps = ps.tile([C, 512], F32, tag="s3", bufs=2)
        nc.tensor.matmul(q_ps[:, 0:w], wsc[:, 0:C],
                         x_sb[:, off:off + w], start=True, stop=True)
        nc.vector.tensor_scalar_add(out=q_sb[:, off:off + w], in0=q_ps[:, 0:w],
                                    scalar1=b_sb[:, 0:1])
        off += w

    k_sb = sb.tile([C, MP], F32R)
    v_sb = sb.tile([C, MP], BF16)
    # zero-pad K columns; memset can't write f32r, so multiply-by-zero
    nc.vector.tensor_scalar_mul(out=k_sb[:, M:MP], in0=wq_t[:, 0:MP - M], scalar1=0.0)
    nc.vector.memset(v_sb[:, M:MP], 0.0)
    for j in range(4):
        lo = j * 432
        k_ps = ps.tile([C, 512], F32, tag="s3", bufs=2)
        nc.tensor.matmul(k_ps[:, 0:432], wsc[:, C:2 * C],
                         xps[:, lo:lo + 432], start=True, stop=True)
        nc.vector.tensor_scalar_add(out=k_sb[:, lo:lo + 432], in0=k_ps[:, 0:432],
                                    scalar1=b_sb[:, 1:2])
        v_ps = ps.tile([C, 512], F32, tag="s3", bufs=2)
        nc.tensor.matmul(v_ps[:, 0:432], wsc[:, 2 * C:3 * C],
                         xps[:, lo:lo + 432], start=True, stop=True)
        nc.vector.tensor_scalar_add(out=v_sb[:, lo:lo + 432], in0=v_ps[:, 0:432],
                                    scalar1=b_sb[:, 2:3])

    # ---------------- V^T (per 128-wide m-tile) ----------------
    vT = sb.tile([C, NMT, C], BF16)
    for mt in range(NMT):
        vt_ps = ps.tile([C, C], BF16, tag=("sums" if mt % 2 else "acc"))
        nc.tensor.transpose(vt_ps, v_sb[:, mt * C:(mt + 1) * C], ident_t)
        nc.vector.tensor_copy(out=vT[:, mt, :], in_=vt_ps)

    # ---------------- attention + proj + residual, per n-block ----------------
    pairs = [(mt, h) for mt in range(NMT) for h in range(NH)]
    groups = [pairs[i:i + 3] for i in range(0, len(pairs), 3)]
    n0 = 0
    for nb in BLOCKS:
        o_ps = ps.tile([C, 512], F32, tag="acc")
        sm_ps = ps.tile([C, 512], F32, tag="sums")
        for grp in groups:
            s3 = ps.tile([C, 3, 512], F32, tag="s3", bufs=2)
            for j, (mt, h) in enumerate(grp):
                nc.tensor.matmul(
                    s3[:, j, 0:nb],
                    k_sb[HD * h:HD * (h + 1), mt * C:(mt + 1) * C],
                    q_sb[HD * h:HD * (h + 1), n0:n0 + nb],
                    start=True, stop=True, tile_position=(HD * h, 0))
            pt = ptp.tile([C, 3, 512], BF16, tag="pt")
            g = len(grp)
            nc.scalar.activation(out=pt[:, 0:g, 0:nb], in_=s3[:, 0:g, 0:nb], func=AF.Exp)
            for j, (mt, h) in enumerate(grp):
                # The 4 head strips accumulate in disjoint 32-partition ranges
                # of one shared bank; the sim's group checker is partition-base
                # agnostic, so it must be skipped here.
                nc.tensor.matmul(
                    o_ps[HD * h:HD * (h + 1), 0:nb],
                    vT[:, mt, HD * h:HD * (h + 1)],
                    pt[:, j, 0:nb],
                    start=(mt == 0), stop=(mt == NMT - 1), tile_position=(0, HD * h),
                    skip_group_check=True)
                nc.tensor.matmul(
                    sm_ps[HD * h:HD * (h + 1), 0:nb],
                    ones_t[:, :],
                    pt[:, j, 0:nb],
                    start=(mt == 0), stop=(mt == NMT - 1), tile_position=(0, HD * h),
                    skip_group_check=True)
        # softmax denominator: subtract the (MP - M) padded exp(0)=1 keys
        sm_sb = stg.tile([C, 512], F32, tag="sm")
        nc.vector.tensor_scalar_add(out=sm_sb[:, 0:nb], in0=sm_ps[:, 0:nb],
                                    scalar1=float(M - MP))
        rs = stg.tile([C, 512], F32, tag="rs")
        nc.vector.reciprocal(out=rs[:, 0:nb], in_=sm_sb[:, 0:nb])
        o1 = stg.tile([C, 512], F32R, tag="o1")
        nc.vector.tensor_tensor(out=o1[:, 0:nb], in0=o_ps[:, 0:nb], in1=rs[:, 0:nb],
                                op=ALU.mult)
        z_ps = ps.tile([C, 512], F32, tag="acc")
        nc.tensor.matmul(z_ps[:, 0:nb], wp_t, o1[:, 0:nb],
                         start=True, stop=True)
        zo = stg.tile([C, 512], F32, tag="zo")
        nc.vector.tensor_scalar_add(out=zo[:, 0:nb], in0=z_ps[:, 0:nb], scalar1=pb_t)
        nc.vector.tensor_tensor(out=zo[:, 0:nb], in0=zo[:, 0:nb], in1=x_sb[:, n0:n0 + nb],
                                op=ALU.add)
        dma.dma_start(out=out[:, n0:n0 + nb], in_=zo[:, 0:nb])
        n0 += nb


def build_nc(repeats=1):
    nc = Bacc(trn_type="TRN2")
    ins = (
        nc.declare_dram_parameter("x", [C, SP], F32R, False),
        nc.declare_dram_parameter("wqkv", [C, 3 * C], F32R, False),
        nc.declare_dram_parameter("bqkv", [C, 3], F32, False),
        nc.declare_dram_parameter("wp", [C, C], F32R, False),
        nc.declare_dram_parameter("pb", [C, 1], F32, False),
        nc.declare_dram_parameter("gnw", [C, 1], F32, False),
        nc.declare_dram_parameter("gnb", [C, 1], F32, False),
        nc.declare_dram_parameter("gsum", [C, 8], F32R, False),
        nc.declare_dram_parameter("gbr", [8, C], F32R, False),
        nc.declare_dram_parameter("ident", [C, C], BF16, False),
        nc.declare_dram_parameter("ones", [C, HD], BF16, False),
    )
    outs = [nc.declare_dram_parameter(f"out{r}" if r else "out", [C, NQ], F32, True)
            for r in range(repeats)]
    with tile.TileContext(nc) as tc:
        for r in range(repeats):
            with ExitStack() as ctx:
                _body(nc, ctx, tc, ins + (outs[r],))
    nc.finalize()
    return nc


def get_nc(repeats=1):
    key = ("nc", repeats)
    if key not in _CACHE:
        _CACHE[key] = build_nc(repeats)
    return _CACHE[key]


def make_in_maps(x, gn_w, gn_b, qkv_w, qkv_b, proj_w, proj_b):
    x = np.asarray(x, np.float32)
    B = x.shape[0]
    scale = HD ** -0.5
    wq = np.array(qkv_w, np.float32).T.copy()            # [C, 3C]
    wq[:, 0:C] *= scale
    bq = np.array(qkv_b, np.float32).reshape(3, C).T.copy()  # [C, 3]
    bq[:, 0] *= scale
    wpt = np.array(proj_w, np.float32).T.copy()          # [C, C]
    pbv = np.array(proj_b, np.float32).reshape(C, 1)
    gnwv = np.array(gn_w, np.float32).reshape(C, 1)
    gnbv = np.array(gn_b, np.float32).reshape(C, 1)
    gsum = np.zeros((C, 8), np.float32)
    gsum[np.arange(C), np.arange(C) // 16] = 1.0 / 16.0
    gbr = np.zeros((8, C), np.float32)
    gbr[np.arange(C) // 16, np.arange(C)] = 1.0
    ident = np.eye(C, dtype=ml_dtypes.bfloat16)
    ones = np.ones((C, HD), dtype=ml_dtypes.bfloat16)
    xf = x.reshape(B, C, SP)
    in_maps = []
    for core in range(8):
        b, qd = core // 4, core % 4
        xr = np.ascontiguousarray(np.roll(xf[b], -qd * NQ, axis=1))
        in_maps.append(dict(x=xr, wqkv=wq, bqkv=bq, wp=wpt, pb=pbv, gnw=gnwv,
                            gnb=gnbv, gsum=gsum, gbr=gbr, ident=ident, ones=ones))
    return in_maps


def assemble(results, shape):
    B = shape[0]
    out = np.empty((B, C, SP), np.float32)
    for core in range(8):
        b, qd = core // 4, core % 4
        out[b][:, qd * NQ:(qd + 1) * NQ] = results[core]["out"]
    return out.reshape(shape)


def run(in_maps, trace=False):
    return run_bass_kernel_spmd(get_nc(), in_maps, list(range(8)), trace=trace)


def kernel(x, gn_w, gn_b, qkv_w, qkv_b, proj_w, proj_b):
    in_maps = make_in_maps(x, gn_w, gn_b, qkv_w, qkv_b, proj_w, proj_b)
    res = run(in_maps)
    return assemble(res.results, np.asarray(x).shape)


# revision 9
# speedup vs baseline: 1.3012x; 1.3012x over previous
"""Trainium2 Bass kernel for a 3D AttentionBlock:
GroupNorm -> 1x1x1-conv QKV -> (2x2x2 avg-pooled K/V) attention -> proj -> residual.

SPMD across 8 NeuronCores: core = (batch b, spatial quarter). Each core computes
the full block for 3456 of the 13824 spatial positions of one batch element; the
pooled K/V (1728 positions) are computed redundantly per core from the full x[b].
No cross-core communication.

A host-side np.roll of x[b] along the flattened spatial dim by the quarter offset
(a whole number of h-plane pairs) makes the program SPMD-uniform: every core's
program processes query columns [0, 3456). GroupNorm stats are permutation
invariant, the 2x2x2 pooling structure is preserved by the 6-plane rotation, and
softmax/attention are invariant to the induced permutation of key positions.

Algebraic folds:
 - GroupNorm affine (data-dependent per-channel scale s_c / shift t_c) is folded
   into the QKV weights on device: W' = W .* s_c (per input channel), b' = W@t + b.
 - avg-pooling commutes with the 1x1 conv: K/V are computed from pooled(x).
   The 1/8 pool mean is folded into the K/V weight scaling.
 - the attention scale (hd^-0.5) is folded into W_q/b_q on the host.
 - K is zero-padded 1728 -> 1792 (14 full 128-wide m-tiles); this adds exp(0)=1
   to every softmax denominator 64 times, which is subtracted exactly; padded V
   rows are zero so the AV matmul is unaffected.

PE usage: scores are computed transposed, S^T[m,n] = k^T q, with the 4 heads
row-tiled (tile_position=(32h,0), K=32 each). exp runs on ScalarE directly from
PSUM in 3-bank groups (this is the kernel's bottleneck: ~24M exps per core).
AV and the softmax-denominator matmuls are col-tiled per head
(tile_position=(0,32h)) accumulating over the 14 m-tiles in single PSUM banks.
Matmuls use float32r (1 cycle/row); probabilities and V are bf16.
"""

import numpy as np
import ml_dtypes
from contextlib import ExitStack

import concourse.bass as bass
import concourse.tile as tile
from concourse import mybir
from concourse.bacc import Bacc
from concourse.bass_utils import run_bass_kernel_spmd

F32 = mybir.dt.float32
F32R = mybir.dt.float32r
BF16 = mybir.dt.bfloat16
AF = mybir.ActivationFunctionType
ALU = mybir.AluOpType

C = 128            # channels
SP = 13824         # 24^3 spatial
NQ = SP // 4       # 3456 query columns per core
M = 1728           # pooled 12^3
MP = 1792          # padded to 14*128
NMT = MP // 128    # 14 m-tiles
NH = 4             # heads
HD = 32            # head dim
EPS = 1e-5
BLOCKS = [512] * 6 + [384]   # n-blocks covering NQ
XCH = 8                      # x DMA chunks
XCW = SP // XCH              # 1728 cols per chunk

_CACHE = {}


def _body(nc, ctx, tc, dram):
    x, wqkv, bqkv, wp, pb, gnw, gnb, gsum, gbr, ident, sel4, out = dram

    const = ctx.enter_context(tc.tile_pool(name="const", bufs=1))
    sb = ctx.enter_context(tc.tile_pool(name="sb", bufs=1))
    work = ctx.enter_context(tc.tile_pool(name="work", bufs=2))
    ptp = ctx.enter_context(tc.tile_pool(name="ptp", bufs=3))
    stg = ctx.enter_context(tc.tile_pool(name="stg", bufs=2))
    ps = ctx.enter_context(tc.tile_pool(name="ps", bufs=1, space="PSUM"))

    dma = nc.default_dma_engine

    # ---------------- constants ----------------
    wq_t = const.tile([C, 3 * C], F32R)
    dma.dma_start(out=wq_t, in_=wqkv[:, :])
    bq_t = const.tile([C, 3], F32)
    dma.dma_start(out=bq_t, in_=bqkv[:, :])
    wp_t = const.tile([C, C], F32R)
    dma.dma_start(out=wp_t, in_=wp[:, :])
    pb_t = const.tile([C, 1], F32)
    dma.dma_start(out=pb_t, in_=pb[:, :])
    gnw_t = const.tile([C, 1], F32)
    dma.dma_start(out=gnw_t, in_=gnw[:, :])
    gnb_t = const.tile([C, 1], F32)
    dma.dma_start(out=gnb_t, in_=gnb[:, :])
    gsum_t = const.tile([C, 8], F32R)
    dma.dma_start(out=gsum_t, in_=gsum[:, :])
    gbr_t = const.tile([8, C], F32R)
    dma.dma_start(out=gbr_t, in_=gbr[:, :])
    ident_t = const.tile([C, C], BF16)
    dma.dma_start(out=ident_t, in_=ident[:, :])
    sel4_t = const.tile([1, NH * C], F32R)
    dma.dma_start(out=sel4_t, in_=sel4[:, :])
    eps_t = const.tile([C, 1], F32)
    nc.vector.memset(eps_t, EPS)

    # ---------------- load x; per-channel stats; pooling ----------------
    x_sb = sb.tile([C, SP], F32R)
    stats = sb.tile([C, 32, 6], F32)
    xps = sb.tile([C, M], F32R)  # pooled *sums* (x8 of the mean)
    for ch in range(XCH):
        dma.dma_start(out=x_sb[:, ch * XCW:(ch + 1) * XCW],
                      in_=x[:, ch * XCW:(ch + 1) * XCW])
        for j in range(4):
            lo = ch * XCW + j * 432
            nc.vector.bn_stats(out=stats[:, ch * 4 + j, :], in_=x_sb[:, lo:lo + 432])
    for st in range(4):  # each step pools 6 h-planes (two DMA chunks)
        base = st * 3456
        xv = x_sb[:, base:base + 3456].rearrange(
            "p (h w d t) -> p h w d t", h=6, w=24, d=12, t=2)
        t1 = work.tile([C, 6, 24, 12], F32, tag="t1")
        nc.vector.tensor_tensor(out=t1, in0=xv[:, :, :, :, 0], in1=xv[:, :, :, :, 1],
                                op=ALU.add)
        t1v = t1.rearrange("p h (w t) d -> p h w t d", t=2)
        t2 = work.tile([C, 6, 12, 12], F32, tag="t2")
        nc.vector.tensor_tensor(out=t2, in0=t1v[:, :, :, 0, :], in1=t1v[:, :, :, 1, :],
                                op=ALU.add)
        t2v = t2.rearrange("p (h t) w d -> p h t w d", t=2)
        ov = xps[:, st * 432:(st + 1) * 432].rearrange("p (h w d) -> p h w d", h=3, w=12)
        nc.vector.tensor_tensor(out=ov, in0=t2v[:, :, 0, :, :], in1=t2v[:, :, 1, :, :],
                                op=ALU.add)

    # ---------------- GroupNorm stats -> per-channel scale/shift ----------------
    mv = sb.tile([C, 2], F32)
    nc.vector.bn_aggr(out=mv, in_=stats)
    m12 = sb.tile([C, 2], F32R)          # [mean_c, E[x^2]_c]
    nc.vector.tensor_copy(out=m12[:, 0:1], in_=mv[:, 0:1])
    nc.vector.tensor_tensor(out=m12[:, 1:2], in0=mv[:, 0:1], in1=mv[:, 0:1], op=ALU.mult)
    nc.vector.tensor_tensor(out=m12[:, 1:2], in0=m12[:, 1:2], in1=mv[:, 1:2], op=ALU.add)
    g_ps = ps.tile([8, 2], F32, tag="av", bufs=2)
    nc.tensor.matmul(g_ps, gsum_t.bitcast(F32), m12.bitcast(F32), start=True, stop=True)
    g_sb = sb.tile([8, 2], F32R)
    nc.vector.tensor_copy(out=g_sb, in_=g_ps)
    bc_ps = ps.tile([C, 2], F32, tag="av", bufs=2)
    nc.tensor.matmul(bc_ps, gbr_t.bitcast(F32), g_sb.bitcast(F32), start=True, stop=True)
    bc = sb.tile([C, 2], F32)           # [mu_g, E_g[x^2]] broadcast to channels
    nc.vector.tensor_copy(out=bc, in_=bc_ps)
    var_t = sb.tile([C, 1], F32)
    nc.vector.tensor_tensor(out=var_t, in0=bc[:, 0:1], in1=bc[:, 0:1], op=ALU.mult)
    nc.vector.tensor_tensor(out=var_t, in0=bc[:, 1:2], in1=var_t, op=ALU.subtract)
    sd_t = sb.tile([C, 1], F32)
    nc.scalar.activation(out=sd_t, in_=var_t, func=AF.Sqrt, bias=eps_t)
    r_t = sb.tile([C, 1], F32)
    nc.vector.reciprocal(out=r_t, in_=sd_t)
    s_t = sb.tile([C, 1], F32)          # s_c = gamma_c * rsqrt(var+eps)
    nc.vector.tensor_tensor(out=s_t, in0=r_t, in1=gnw_t, op=ALU.mult)
    s8_t = sb.tile([C, 1], F32)         # s_c / 8 (pool mean fold)
    nc.vector.tensor_scalar_mul(out=s8_t, in0=s_t, scalar1=0.125)
    tt_t = sb.tile([C, 1], F32R)         # t_c = beta_c - mu_c * s_c
    nc.vector.tensor_tensor(out=tt_t, in0=bc[:, 0:1], in1=s_t, op=ALU.mult)
    nc.vector.tensor_tensor(out=tt_t, in0=gnb_t, in1=tt_t, op=ALU.subtract)

    # ---------------- fold GN into QKV weights / biases ----------------
    wsc = sb.tile([C, 3 * C], F32R)
    nc.vector.tensor_scalar_mul(out=wsc[:, 0:C], in0=wq_t[:, 0:C], scalar1=s_t)
    nc.vector.tensor_scalar_mul(out=wsc[:, C:3 * C], in0=wq_t[:, C:3 * C], scalar1=s8_t)
    b_ps = ps.tile([C, 3], F32, tag="av", bufs=2)
    for j in range(3):
        nc.tensor.matmul(b_ps[:, j:j + 1], wq_t[:, j * C:(j + 1) * C].bitcast(F32),
                         tt_t.bitcast(F32), start=True, stop=True)
    b_sb = sb.tile([C, 3], F32)
    nc.vector.tensor_tensor(out=b_sb, in0=b_ps, in1=bq_t, op=ALU.add)

    # ---------------- QKV ----------------
    q_sb = sb.tile([C, NQ], F32R)
    off = 0
    for w in BLOCKS:
        q_ps = ps.tile([C, 512], F32, tag="s3", bufs=2)
        nc.tensor.matmul(q_ps[:, 0:w], wsc[:, 0:C],
                         x_sb[:, off:off + w], start=True, stop=True)
        nc.vector.tensor_scalar_add(out=q_sb[:, off:off + w], in0=q_ps[:, 0:w],
                                    scalar1=b_sb[:, 0:1])
        off += w

    k_sb = sb.tile([C, MP], F32R)
    v_sb = sb.tile([C, MP], BF16)
    # zero-pad K columns; memset can't write f32r, so multiply-by-zero
    nc.vector.tensor_scalar_mul(out=k_sb[:, M:MP], in0=wq_t[:, 0:MP - M], scalar1=0.0)
    nc.vector.memset(v_sb[:, M:MP], 0.0)
    for j in range(4):
        lo = j * 432
        k_ps = ps.tile([C, 512], F32, tag="s3", bufs=2)
        nc.tensor.matmul(k_ps[:, 0:432], wsc[:, C:2 * C],
                         xps[:, lo:lo + 432], start=True, stop=True)
        nc.vector.tensor_scalar_add(out=k_sb[:, lo:lo + 432], in0=k_ps[:, 0:432],
                                    scalar1=b_sb[:, 1:2])
        v_ps = ps.tile([C, 512], F32, tag="s3", bufs=2)
        nc.tensor.matmul(v_ps[:, 0:432], wsc[:, 2 * C:3 * C],
                         xps[:, lo:lo + 432], start=True, stop=True)
        nc.vector.tensor_scalar_add(out=v_sb[:, lo:lo + 432], in0=v_ps[:, 0:432],
                                    scalar1=b_sb[:, 2:3])

    # ---------------- V^T (per 128-wide m-tile) ----------------
    # vTa[:, mt, h, 0:32] = V^T for head h (m-tile mt); col 32 = 1.0 so the AV
    # matmul also emits the softmax denominator as a 33rd output row.
    vTa = sb.tile([C, NMT, NH, 33], BF16)
    nc.vector.memset(vTa[:, :, :, 32:33], 1.0)
    for mt in range(NMT):
        vt_ps = ps.tile([C, C], BF16, tag="av", bufs=2)
        nc.tensor.transpose(vt_ps, v_sb[:, mt * C:(mt + 1) * C], ident_t)
        for h in range(NH):
            nc.vector.tensor_copy(out=vTa[:, mt, h, 0:32],
                                  in_=vt_ps[:, HD * h:HD * (h + 1)])

    # ---------------- attention + proj + residual, per n-block ----------------
    pairs = [(mt, h) for mt in range(NMT) for h in range(NH)]
    groups = [pairs[i:i + 3] for i in range(0, len(pairs), 3)]
    n0 = 0
    for nb in BLOCKS:
        # two AV accumulator banks; heads (0,1) at partition bases (0,64) of
        # bank A, heads (2,3) likewise in bank B. Rows base+0:32 = O_h, row
        # base+32 = softmax denominator (ones column of vTa).
        oa = ps.tile([C, 512], F32, tag="av", bufs=2)
        ob = ps.tile([C, 512], F32, tag="av", bufs=2)
        banks = (oa, oa, ob, ob)
        for grp in groups:
            s3 = ps.tile([C, 3, 512], F32, tag="s3", bufs=2)
            for j, (mt, h) in enumerate(grp):
                nc.tensor.matmul(
                    s3[:, j, 0:nb],
                    k_sb[HD * h:HD * (h + 1), mt * C:(mt + 1) * C],
                    q_sb[HD * h:HD * (h + 1), n0:n0 + nb],
                    start=True, stop=True, tile_position=(HD * h, 0))
            pt = ptp.tile([C, 3, 512], BF16, tag="pt")
            g = len(grp)
            nc.scalar.activation(out=pt[:, 0:g, 0:nb], in_=s3[:, 0:g, 0:nb], func=AF.Exp)
            for j, (mt, h) in enumerate(grp):
                # Two 33-row accumulation groups share each bank at disjoint
                # partition bases {0, 64}; the sim's group checker is
                # partition-base agnostic, so it must be skipped here.
                base = 64 * (h % 2)
                nc.tensor.matmul(
                    banks[h][base:base + 33, 0:nb],
                    vTa[:, mt, h, :],
                    pt[:, j, 0:nb],
                    start=(mt == 0), stop=(mt == NMT - 1), tile_position=(0, base),
                    skip_group_check=True)
        # copy the 4 denominator rows to SBUF, then broadcast each to its
        # head's 32 rows with accumulating K=1 selector matmuls
        s4 = stg.tile([1, NH, 512], F32R, tag="s4")
        for h in range(NH):
            base = 64 * (h % 2)
            nc.vector.tensor_copy(out=s4[0:1, h, 0:nb],
                                  in_=banks[h][base + 32:base + 33, 0:nb])
        rs_ps = ps.tile([C, 512], F32, tag="s3", bufs=2)
        for h in range(NH):
            nc.tensor.matmul(rs_ps[:, 0:nb], sel4_t[0:1, h * C:(h + 1) * C],
                             s4[0:1, h, 0:nb],
                             start=(h == 0), stop=(h == NH - 1))
        # subtract the (MP - M) padded exp(0)=1 keys, then reciprocal
        sm_sb = stg.tile([C, 512], F32, tag="sm")
        nc.vector.tensor_scalar_add(out=sm_sb[:, 0:nb], in0=rs_ps[:, 0:nb],
                                    scalar1=float(M - MP))
        rs = stg.tile([C, 512], F32, tag="rs")
        nc.vector.reciprocal(out=rs[:, 0:nb], in_=sm_sb[:, 0:nb])
        o1 = stg.tile([C, 512], F32R, tag="o1")
        for h in range(NH):
            base = 64 * (h % 2)
            nc.vector.tensor_tensor(out=o1[HD * h:HD * (h + 1), 0:nb],
                                    in0=banks[h][base:base + 32, 0:nb],
                                    in1=rs[HD * h:HD * (h + 1), 0:nb], op=ALU.mult)
        z_ps = ps.tile([C, 512], F32, tag="av", bufs=2)
        nc.tensor.matmul(z_ps[:, 0:nb], wp_t, o1[:, 0:nb],
                         start=True, stop=True)
        zo = stg.tile([C, 512], F32, tag="zo")
        nc.vector.tensor_scalar_add(out=zo[:, 0:nb], in0=z_ps[:, 0:nb], scalar1=pb_t)
        nc.vector.tensor_tensor(out=zo[:, 0:nb], in0=zo[:, 0:nb], in1=x_sb[:, n0:n0 + nb],
                                op=ALU.add)
        dma.dma_start(out=out[:, n0:n0 + nb], in_=zo[:, 0:nb])
        n0 += nb


def build_nc(repeats=1):
    nc = Bacc(trn_type="TRN2")
    ins = (
        nc.declare_dram_parameter("x", [C, SP], F32R, False),
        nc.declare_dram_parameter("wqkv", [C, 3 * C], F32R, False),
        nc.declare_dram_parameter("bqkv", [C, 3], F32, False),
        nc.declare_dram_parameter("wp", [C, C], F32R, False),
        nc.declare_dram_parameter("pb", [C, 1], F32, False),
        nc.declare_dram_parameter("gnw", [C, 1], F32, False),
        nc.declare_dram_parameter("gnb", [C, 1], F32, False),
        nc.declare_dram_parameter("gsum", [C, 8], F32R, False),
        nc.declare_dram_parameter("gbr", [8, C], F32R, False),
        nc.declare_dram_parameter("ident", [C, C], BF16, False),
        nc.declare_dram_parameter("sel4", [1, NH * C], F32R, False),
    )
    outs = [nc.declare_dram_parameter(f"out{r}" if r else "out", [C, NQ], F32, True)
            for r in range(repeats)]
    with tile.TileContext(nc) as tc:
        for r in range(repeats):
            with ExitStack() as ctx:
                _body(nc, ctx, tc, ins + (outs[r],))
    nc.finalize()
    return nc


def get_nc(repeats=1):
    key = ("nc", repeats)
    if key not in _CACHE:
        _CACHE[key] = build_nc(repeats)
    return _CACHE[key]


def make_in_maps(x, gn_w, gn_b, qkv_w, qkv_b, proj_w, proj_b):
    x = np.asarray(x, np.float32)
    B = x.shape[0]
    scale = HD ** -0.5
    wq = np.array(qkv_w, np.float32).T.copy()            # [C, 3C]
    wq[:, 0:C] *= scale
    bq = np.array(qkv_b, np.float32).reshape(3, C).T.copy()  # [C, 3]
    bq[:, 0] *= scale
    wpt = np.array(proj_w, np.float32).T.copy()          # [C, C]
    pbv = np.array(proj_b, np.float32).reshape(C, 1)
    gnwv = np.array(gn_w, np.float32).reshape(C, 1)
    gnbv = np.array(gn_b, np.float32).reshape(C, 1)
    gsum = np.zeros((C, 8), np.float32)
    gsum[np.arange(C), np.arange(C) // 16] = 1.0 / 16.0
    gbr = np.zeros((8, C), np.float32)
    gbr[np.arange(C) // 16, np.arange(C)] = 1.0
    ident = np.eye(C, dtype=ml_dtypes.bfloat16)
    sel4 = np.zeros((4, C), np.float32)
    sel4[np.arange(C) // HD, np.arange(C)] = 1.0
    sel4 = sel4.reshape(1, 4 * C)
    xf = x.reshape(B, C, SP)
    in_maps = []
    for core in range(8):
        b, qd = core // 4, core % 4
        xr = np.ascontiguousarray(np.roll(xf[b], -qd * NQ, axis=1))
        in_maps.append(dict(x=xr, wqkv=wq, bqkv=bq, wp=wpt, pb=pbv, gnw=gnwv,
                            gnb=gnbv, gsum=gsum, gbr=gbr, ident=ident, sel4=sel4))
    return in_maps


def assemble(results, shape):
    B = shape[0]
    out = np.empty((B, C, SP), np.float32)
    for core in range(8):
        b, qd = core // 4, core % 4
        out[b][:, qd * NQ:(qd + 1) * NQ] = results[core]["out"]
    return out.reshape(shape)


def run(in_maps, trace=False):
    return run_bass_kernel_spmd(get_nc(), in_maps, list(range(8)), trace=trace)


def kernel(x, gn_w, gn_b, qkv_w, qkv_b, proj_w, proj_b):
    in_maps = make_in_maps(x, gn_w, gn_b, qkv_w, qkv_b, proj_w, proj_b)
    res = run(in_maps)
    return assemble(res.results, np.asarray(x).shape)
